# revision 31
# baseline (speedup 1.0000x reference)
"""Trainium2 Bass kernel for nn_CHESHIRE (hypergraph GNN message passing).

Strategy (hyperedge-parallel across the 8 cores):
  * Clique Laplacian has the closed form lap(v) = (v - gsum(v))/7, so the
    K=3 Chebyshev conv collapses to z_j = (A.x_j) @ Wx + w8 @ WC + c_const
    with host-folded weight combos (A, w8 per-hyperedge GraphNorm terms).
  * The encoder output [x || x^2] lives in SBUF as a node-major token table;
    incidence rows are fetched feature-major with ONE transposing SBUF-source
    dma_gather per 512-edge block (4096 descriptors amortize the ~1us SWDGE
    fixed cost, and the transpose removes all PE transpose traffic).
  * Per-edge sums (g8/q8) use a single accumulating identity matmul with a
    stride-0 revisit output AP; per-edge C is accumulated straight into the
    cheb PSUM the same way, so PSUM egress is a scalar-engine activation with
    a per-partition bias and the DVE never reads PSUM for the z path.
  * Max/min poolings are fp16 tensor_tensor trees on DVE; ssq pooling is
    another revisit matmul.
"""

import sys

sys.path.insert(0, "/opt/trn_rl_repo")

import numpy as np

import concourse.bacc as bacc
import concourse.bass as bass
import concourse.mybir as mybir
from concourse import tile
from concourse.bass_utils import run_bass_kernel_spmd

F16 = mybir.dt.float16
F32 = mybir.dt.float32
I16 = mybir.dt.int16
AF = mybir.ActivationFunctionType
OP = mybir.AluOpType

# Problem constants (hardcoded per contract).
N, F, EMB, CONV = 2000, 256, 128, 128
E, S = 20000, 8
NCORES = 8
ECORE = E // NCORES          # 2500
EPAD = 2560                  # padded per-core edge count
NBLK = 5
L = EPAD // NBLK             # 512 edges per block
NIDX = S * L                 # 4096 gathered rows per block
NPAD = 2048                  # padded node count (16 ranks of 128)
RANKS = NPAD // 128
EPS = 1e-5

_CACHE = {}


def _build_program():
    nc = bacc.Bacc(None, target_bir_lowering=False, debug=False)

    featT_d = nc.dram_tensor("featT", [F, NPAD], F16, kind="ExternalInput")
    wenc_d = nc.dram_tensor("wenc", [F, EMB], F16, kind="ExternalInput")
    benc_d = nc.dram_tensor("benc", [1, EMB], F16, kind="ExternalInput")
    wx_d = nc.dram_tensor("wx", [EMB, CONV], F16, kind="ExternalInput")
    wc_d = nc.dram_tensor("wc", [EMB, CONV], F16, kind="ExternalInput")
    wo_d = nc.dram_tensor("wo", [CONV, 2], F16, kind="ExternalInput")
    eyef_d = nc.dram_tensor("eyef", [128, 128], F16, kind="ExternalInput")
    vecs_d = nc.dram_tensor("vecs", [128, 8], F32, kind="ExternalInput")
    idx_d = nc.dram_tensor("idx16", [128, NBLK * NIDX // 16], I16,
                           kind="ExternalInput")
    yout_d = nc.dram_tensor("yout", [EPAD], F32, kind="ExternalOutput")

    with tile.TileContext(nc) as tc:
        with (
            tc.tile_pool(name="weights", bufs=1) as wpool,
            tc.tile_pool(name="smalls", bufs=2) as spool,
            tc.tile_pool(name="gath", bufs=2) as gpool,
            tc.tile_pool(name="big", bufs=2) as bigp,
            tc.tile_pool(name="psV", bufs=2, space="PSUM") as psV,
            tc.tile_pool(name="psG", bufs=1, space="PSUM") as psG,
        ):
            # ---- load weights / tables ----
            featT0 = wpool.tile([128, NPAD], F16, tag="featT0")
            featT1 = wpool.tile([128, NPAD], F16, tag="featT1")
            nc.sync.dma_start(featT0[:], featT_d[0:128, :])
            nc.sync.dma_start(featT1[:], featT_d[128:256, :])
            wenc0 = wpool.tile([128, EMB], F16, tag="wenc0")
            wenc1 = wpool.tile([128, EMB], F16, tag="wenc1")
            nc.sync.dma_start(wenc0[:], wenc_d[0:128, :])
            nc.sync.dma_start(wenc1[:], wenc_d[128:256, :])
            benc = wpool.tile([1, EMB], F16, tag="benc")
            nc.sync.dma_start(benc[:], benc_d[:])
            wx = wpool.tile([EMB, CONV], F16, tag="wx")
            nc.sync.dma_start(wx[:], wx_d[:])
            wc = wpool.tile([EMB, CONV], F16, tag="wc")
            nc.sync.dma_start(wc[:], wc_d[:])
            wo = wpool.tile([CONV, 2], F16, tag="wo")
            nc.sync.dma_start(wo[:], wo_d[:])
            vecs = wpool.tile([128, 8], F32, tag="vecs")
            nc.sync.dma_start(vecs[:], vecs_d[:])
            idx = wpool.tile([128, NBLK * NIDX // 16], I16, tag="idx")
            nc.sync.dma_start(idx[:], idx_d[:])
            ones = wpool.tile([1, 128], F16, tag="ones")
            nc.vector.memset(ones[:], 1.0)

            c2v = vecs[:, 0:1]      # (2s - s^2)/8 per EMB feature
            wgv = vecs[:, 1:2]      # gn_weight
            cconv = vecs[:, 3:4]    # c_const (+cheb_b) per CONV feature
            boutv = vecs[0:1, 4:5]  # b_out scalar
            tinyv = vecs[:, 5:6]    # 1e-30
            epsv = vecs[:, 6:7]     # EPS

            # ---- encoder -> node-major token table in SBUF ----
            # table[p, r*128 + e] = clip(xenc)[node r*128+p, e]
            table = wpool.tile([128, RANKS * EMB], F16, tag="table")
            for g in range(4):
                ep = psG.tile([128, 512], F32, tag="sp", name=f"ep{g}")
                for t4 in range(4):
                    t = 4 * g + t4
                    sl = bass.ts(t, 128)
                    out = ep[:, bass.ts(t4, 128)]
                    nc.tensor.matmul(out, featT0[:, sl], wenc0[:],
                                     start=True, stop=False)
                    nc.tensor.matmul(out, featT1[:, sl], wenc1[:],
                                     start=False, stop=False)
                    nc.tensor.matmul(out, ones[:], benc[:],
                                     start=False, stop=True)
                nc.vector.tensor_scalar(
                    table[:, bass.ts(g, 512)], ep[:],
                    1.0, -1.0, op0=OP.min, op1=OP.max)

            logit = wpool.tile([1, EPAD], F32, tag="logit")

            for b in range(NBLK):
                # ---- transposed SBUF-source gathers: feature-major x rows.
                # One gather is capped at ~1000 idxs (ucode idx staging) and
                # ~125 descriptors per direction.
                # xg[p, j*L + e] = x[feat p, node(edge e, member j)]
                xg = gpool.tile([128, S * L], F16, tag="xg")
                off = 0
                for n in (512,) * 8:
                    nc.gpsimd.dma_gather(
                        out_ap=xg[:, off:off + n].unsqueeze(1),
                        in_ap=table[:],
                        idxs_ap=idx[:, b * (NIDX // 16) + off // 16:
                                    b * (NIDX // 16) + (off + n) // 16],
                        num_idxs=n,
                        num_idxs_reg=n,
                        elem_size=EMB,
                        transpose=True,
                        sbuf_tokens_per_rank=128,
                        sbuf_free_dim_per_rank=2 * EMB,
                        sbuf_free_dim_pad_per_rank=0,
                        sbuf_byte_offset=0,
                        queue_num=0,
                    )
                    off += n
                xgx = xg[:].rearrange("p (j e) -> p j e", j=S)
                xsq = bigp.tile([128, S, L], F16, tag="xsq")
                nc.scalar.activation(xsq[:], xgx, AF.Square)

                # ---- per-edge sums over the 8 member planes: fp16
                # add-trees, final level in fp32.
                g1 = bigp.tile([128, 4, L], F16, tag="gq1")
                nc.vector.tensor_tensor(g1[:], xgx[:, 0:4], xgx[:, 4:8],
                                        op=OP.add)
                g2 = spool.tile([128, 2, L], F16, tag="g2")
                nc.vector.tensor_tensor(g2[:], g1[:, 0:2], g1[:, 2:4],
                                        op=OP.add)
                g8s = spool.tile([128, L], F32, tag="g8s")
                nc.vector.tensor_tensor(g8s[:], g2[:, 0], g2[:, 1], op=OP.add)
                q1 = bigp.tile([128, 4, L], F16, tag="q1")
                nc.vector.tensor_tensor(q1[:], xsq[:, 0:4], xsq[:, 4:8],
                                        op=OP.add)
                q2 = spool.tile([128, 2, L], F16, tag="q2")
                nc.vector.tensor_tensor(q2[:], q1[:, 0:2], q1[:, 2:4],
                                        op=OP.add)
                q8s = spool.tile([128, L], F32, tag="q8s")
                nc.vector.tensor_tensor(q8s[:], q2[:, 0], q2[:, 1], op=OP.add)

                # GraphNorm per-hyperedge scale A = gn_w / sqrt(var + eps)
                t1 = spool.tile([128, L], F32, tag="t1")
                nc.scalar.activation(t1[:], g8s[:], AF.Square)
                t2 = spool.tile([128, L], F32, tag="t2")
                nc.vector.tensor_scalar(t2[:], t1[:], c2v, None, op0=OP.mult)
                vx8 = spool.tile([128, L], F32, tag="vx8")
                nc.vector.tensor_tensor(vx8[:], q8s[:], t2[:], op=OP.subtract)
                ex = spool.tile([128, L], F32, tag="ex")
                nc.scalar.activation(ex[:], vx8[:], AF.Abs_reciprocal_sqrt,
                                     scale=0.125, bias=epsv)
                A16 = spool.tile([128, L], F16, tag="A16")
                nc.vector.tensor_scalar(A16[:], ex[:], wgv, None, op0=OP.mult)
                w8 = spool.tile([128, L], F16, tag="w8")
                nc.vector.tensor_tensor(w8[:], A16[:], g8s[:], op=OP.mult)

                # ---- rhs = A (.) x, broadcast A over the 8 member planes
                rhs = bigp.tile([128, S, L], F16, tag="rhs")
                nc.vector.tensor_tensor(
                    rhs[:], xgx, A16[:].unsqueeze(1).broadcast_to([128, S, L]),
                    op=OP.mult)

                # ---- cheb + per-edge C in PSUM; egress with c_const bias
                z = bigp.tile([128, S, L], F16, tag="z")
                for w in range(4):
                    vp = psV.tile([128, 2, L], F32, tag="vp", name=f"vp{b}_{w}")
                    nc.tensor.matmul(vp[:, 0, :], wx[:], rhs[:, 2 * w, :],
                                     start=True, stop=False)
                    nc.tensor.matmul(vp[:, 1, :], wx[:], rhs[:, 2 * w + 1, :],
                                     start=True, stop=False)
                    nc.tensor.matmul(vp[:, 0, :], wc[:], w8[:],
                                     start=False, stop=True)
                    nc.tensor.matmul(vp[:, 1, :], wc[:], w8[:],
                                     start=False, stop=True)
                    nc.scalar.activation(z[:, 2 * w:2 * w + 2, :], vp[:],
                                         AF.Identity, bias=cconv)

                # ---- poolings over the 8 planes (fp16 DVE trees) ----
                mx1 = bigp.tile([128, 4, L], F16, tag="mx1")
                mn1 = bigp.tile([128, 4, L], F16, tag="mn1")
                nc.vector.tensor_tensor(mx1[:], z[:, 0:4, :], z[:, 4:8, :],
                                        op=OP.max)
                nc.vector.tensor_tensor(mn1[:], z[:, 0:4, :], z[:, 4:8, :],
                                        op=OP.min)
                mx2 = spool.tile([128, 2, L], F16, tag="mx2")
                mn2 = spool.tile([128, 2, L], F16, tag="mn2")
                nc.vector.tensor_tensor(mx2[:], mx1[:, 0:2, :], mx1[:, 2:4, :],
                                        op=OP.max)
                nc.vector.tensor_tensor(mn2[:], mn1[:, 0:2, :], mn1[:, 2:4, :],
                                        op=OP.min)
                zmax = spool.tile([128, L], F16, tag="zmax")
                zmin = spool.tile([128, L], F16, tag="zmin")
                nc.vector.tensor_tensor(zmax[:], mx2[:, 0, :], mx2[:, 1, :],
                                        op=OP.max)
                nc.vector.tensor_tensor(zmin[:], mn2[:, 0, :], mn2[:, 1, :],
                                        op=OP.min)
                # rng = clip(zmax) - clip(zmin)
                zmaxc = spool.tile([128, L], F16, tag="zmaxc")
                zminc = spool.tile([128, L], F16, tag="zminc")
                nc.vector.tensor_scalar(zmaxc[:], zmax[:], 1.0, -1.0,
                                        op0=OP.min, op1=OP.max)
                nc.vector.tensor_scalar(zminc[:], zmin[:], 1.0, -1.0,
                                        op0=OP.min, op1=OP.max)
                rng = spool.tile([128, L], F16, tag="rng")
                nc.vector.tensor_tensor(rng[:], zmaxc[:], zminc[:],
                                        op=OP.subtract)

                # ---- ynorm = sqrt(mean_j min(z^2, 1)) ----
                sqm = bigp.tile([128, S, L], F16, tag="rhs", name=f"sqm{b}")
                nc.scalar.activation(sqm[:], z[:], AF.Square)
                nc.vector.tensor_scalar(sqm[:], sqm[:], 1.0, None, op0=OP.min)
                s1 = bigp.tile([128, 4, L], F16, tag="mx1", name=f"s1{b}")
                nc.vector.tensor_tensor(s1[:], sqm[:, 0:4, :], sqm[:, 4:8, :],
                                        op=OP.add)
                s2 = spool.tile([128, 2, L], F16, tag="mx2", name=f"s2{b}")
                nc.vector.tensor_tensor(s2[:], s1[:, 0:2, :], s1[:, 2:4, :],
                                        op=OP.add)
                ssq = spool.tile([128, L], F32, tag="ssq")
                nc.vector.tensor_tensor(ssq[:], s2[:, 0, :], s2[:, 1, :],
                                        op=OP.add)
                r2 = spool.tile([128, L], F32, tag="r2")
                nc.scalar.activation(r2[:], ssq[:], AF.Abs_reciprocal_sqrt,
                                     scale=0.125, bias=tinyv)
                ynorm = spool.tile([128, L], F16, tag="ynorm")
                nc.vector.scalar_tensor_tensor(ynorm[:], ssq[:], 0.125, r2[:],
                                               op0=OP.mult, op1=OP.mult)

                fp = psG.tile([1, L], F32, tag="fp", name=f"fp{b}")
                nc.tensor.matmul(fp[:], wo[:, 0:1], rng[:],
                                 start=True, stop=False)
                nc.tensor.matmul(fp[:], wo[:, 1:2], ynorm[:],
                                 start=False, stop=True)
                nc.scalar.activation(logit[0:1, bass.ts(b, L)], fp[:],
                                     AF.Identity)

            nc.scalar.activation(logit[:], logit[:], AF.Sigmoid, bias=boutv)
            nc.sync.dma_start(yout_d[:].rearrange("(p c) -> p c", p=1),
                              logit[:])

    nc.compile()
    return nc


def _get_program():
    if "nc" not in _CACHE:
        _CACHE["nc"] = _build_program()
    return _CACHE["nc"]


def _host_prep(inputs):
    """Fold weights and stage per-core input maps."""
    f = lambda k: np.asarray(inputs[k], np.float32)
    feature = f("feature")
    W_enc, b_enc = f("W_enc"), f("b_enc")
    gw, gb, gs = f("gn_weight"), f("gn_bias"), f("gn_mean_scale")
    cheb_W = np.asarray(inputs["cheb_W"], np.float64)
    cheb_b = np.asarray(inputs["cheb_b"], np.float64)
    W_out, b_out = f("W_out"), f("b_out")
    hn = np.asarray(inputs["hyperedge_nodes"]).astype(np.int64)

    d = float(S - 1)
    W0, W1, W2 = cheb_W[0], cheb_W[1], cheb_W[2]
    Wx64 = W0 + W1 / d + W2 * ((2.0 - d * d) / (d * d))
    Wg64 = -W1 / d + W2 * (2.0 * (d - 1.0) / (d * d))
    # z_j = (A.x_j) @ Wx + w8 @ WC + c_const,  w8 = A.g8
    WC64 = Wg64 - (gs.astype(np.float64)[:, None] / S) * (Wx64 + S * Wg64)
    c_const = gb.astype(np.float64) @ (Wx64 + S * Wg64) + cheb_b

    featT = np.zeros((F, NPAD), np.float16)
    featT[:, :N] = feature.T.astype(np.float16)
    wenc = W_enc.astype(np.float16)
    benc = b_enc.reshape(1, EMB).astype(np.float16)
    wx16 = Wx64.astype(np.float16)
    wc16 = WC64.astype(np.float16)
    wo16 = np.stack([W_out[:CONV, 0], W_out[CONV:, 0]], axis=1).astype(np.float16)
    eyef = np.eye(128, dtype=np.float16)
    vecs = np.zeros((128, 8), np.float32)
    vecs[:, 0] = (2.0 * gs - gs * gs) / 8.0
    vecs[:, 1] = gw
    vecs[:, 3] = c_const.astype(np.float32)
    vecs[0, 4] = b_out[0]
    vecs[:, 5] = 1e-30
    vecs[:, 6] = EPS

    shared = dict(featT=featT, wenc=wenc, benc=benc, wx=wx16, wc=wc16,
                  wo=wo16, eyef=eyef, vecs=vecs)

    in_maps = []
    for c in range(NCORES):
        base = c * ECORE
        hcol = np.zeros((EPAD, S), np.int16)
        hcol[:ECORE] = hn[base:base + ECORE].astype(np.int16)
        # per-block idx stream (plane-major: i = j*L + e), wrapped in 16
        # partitions; gather chunks slice contiguous column ranges.
        ids = np.transpose(hcol.reshape(NBLK, L, S), (0, 2, 1)).reshape(
            NBLK, NIDX)
        idxw = np.zeros((NBLK, 16, NIDX // 16), np.int16)
        pos = np.arange(NIDX)
        for b in range(NBLK):
            idxw[b, pos % 16, pos // 16] = ids[b]
        idx16 = np.tile(idxw.transpose(1, 0, 2).reshape(16, -1), (8, 1))
        in_maps.append(dict(shared, idx16=idx16))
    return in_maps


def _install_trace_hook():
    """Best-effort NTFF profiling under axon (test/benchmark only)."""
    import types
    ah = sys.modules.get("antenv.axon_hooks")
    if ah is None:
        ah = types.ModuleType("antenv.axon_hooks")
        ah._HOOK = None
        ah.set_axon_ntff_profile_hook = lambda h: setattr(ah, "_HOOK", h)
        ah.get_axon_ntff_profile_hook = lambda: ah._HOOK
        sys.modules["antenv.axon_hooks"] = ah
        import antenv
        antenv.axon_hooks = ah
    if ah.get_axon_ntff_profile_hook() is None:
        from trn_agent_boot.trn_boot import _ntff_profile_via_ctypes
        hook = _ntff_profile_via_ctypes("/opt/axon/libaxon_pjrt.so")
        if hook is not None:
            ah.set_axon_ntff_profile_hook(hook)
    import concourse.bass_utils as bu
    bu.upload_artifacts = lambda tmpdir: f"local:{tmpdir}"


def _run(in_maps, trace=False):
    nc = _get_program()
    if trace:
        _install_trace_hook()
    return run_bass_kernel_spmd(nc, in_maps, list(range(NCORES)), trace=trace)


def kernel(**inputs) -> np.ndarray:
    in_maps = _host_prep(inputs)
    res = _run(in_maps)
    out = np.concatenate([res.results[c]["yout"][:ECORE] for c in range(NCORES)])
    return out.reshape(E, 1).astype(np.float32)


def kernel_traced(**inputs):
    """Like kernel() but returns (output, exec_time_ns) using a profiled run."""
    in_maps = _host_prep(inputs)
    res = _run(in_maps, trace=True)
    out = np.concatenate([res.results[c]["yout"][:ECORE] for c in range(NCORES)])
    return out.reshape(E, 1).astype(np.float32), res.exec_time_ns


# revision 34
# speedup vs baseline: 1.3511x; 1.3511x over previous
"""Trainium2 Bass kernel for nn_CHESHIRE (hypergraph GNN message passing).

Strategy (hyperedge-parallel across the 8 cores):
  * Clique Laplacian has the closed form lap(v) = (v - gsum(v))/7, so the
    K=3 Chebyshev conv collapses to z_j = (A.x_j) @ Wx + w8 @ WC + c_const
    with host-folded weight combos (A, w8 per-hyperedge GraphNorm terms).
  * The encoder output [x || x^2] lives in SBUF as a node-major token table;
    incidence rows are fetched feature-major with ONE transposing SBUF-source
    dma_gather per 512-edge block (4096 descriptors amortize the ~1us SWDGE
    fixed cost, and the transpose removes all PE transpose traffic).
  * Per-edge sums (g8/q8) use a single accumulating identity matmul with a
    stride-0 revisit output AP; per-edge C is accumulated straight into the
    cheb PSUM the same way, so PSUM egress is a scalar-engine activation with
    a per-partition bias and the DVE never reads PSUM for the z path.
  * Max/min poolings are fp16 tensor_tensor trees on DVE; ssq pooling is
    another revisit matmul.
"""

import sys

sys.path.insert(0, "/opt/trn_rl_repo")

import numpy as np

import concourse.bacc as bacc
import concourse.bass as bass
import concourse.mybir as mybir
from concourse import tile
from concourse.bass_utils import run_bass_kernel_spmd

F16 = mybir.dt.float16
F32 = mybir.dt.float32
I16 = mybir.dt.int16
AF = mybir.ActivationFunctionType
OP = mybir.AluOpType

# Problem constants (hardcoded per contract).
N, F, EMB, CONV = 2000, 256, 128, 128
E, S = 20000, 8
NCORES = 8
ECORE = E // NCORES          # 2500
EPAD = 2560                  # padded per-core edge count
NBLK = 5
L = EPAD // NBLK             # 512 edges per block
NIDX = S * L                 # 4096 gathered rows per block
NPAD = 2048                  # padded node count (16 ranks of 128)
RANKS = NPAD // 128
EPS = 1e-5

_CACHE = {}


def _build_program():
    nc = bacc.Bacc(None, target_bir_lowering=False, debug=False,
                   num_swdge_queues=2)

    featT_d = nc.dram_tensor("featT", [F, NPAD], F16, kind="ExternalInput")
    wenc_d = nc.dram_tensor("wenc", [F, EMB], F16, kind="ExternalInput")
    benc_d = nc.dram_tensor("benc", [1, EMB], F16, kind="ExternalInput")
    wx_d = nc.dram_tensor("wx", [EMB, CONV], F16, kind="ExternalInput")
    wc_d = nc.dram_tensor("wc", [EMB, CONV], F16, kind="ExternalInput")
    wo_d = nc.dram_tensor("wo", [CONV, 2], F16, kind="ExternalInput")
    eyef_d = nc.dram_tensor("eyef", [128, 128], F16, kind="ExternalInput")
    vecs_d = nc.dram_tensor("vecs", [128, 8], F32, kind="ExternalInput")
    idx_d = nc.dram_tensor("idx16", [128, NBLK * NIDX // 16], I16,
                           kind="ExternalInput")
    yout_d = nc.dram_tensor("yout", [EPAD], F32, kind="ExternalOutput")

    with tile.TileContext(nc) as tc:
        with (
            tc.tile_pool(name="weights", bufs=1) as wpool,
            tc.tile_pool(name="smalls", bufs=2) as spool,
            tc.tile_pool(name="gath", bufs=2) as gpool,
            tc.tile_pool(name="big", bufs=2) as bigp,
            tc.tile_pool(name="psV", bufs=2, space="PSUM") as psV,
            tc.tile_pool(name="psG", bufs=1, space="PSUM") as psG,
        ):
            # ---- load weights / tables ----
            featT0 = wpool.tile([128, NPAD], F16, tag="featT0")
            featT1 = wpool.tile([128, NPAD], F16, tag="featT1")
            nc.sync.dma_start(featT0[:], featT_d[0:128, :])
            nc.sync.dma_start(featT1[:], featT_d[128:256, :])
            wenc0 = wpool.tile([128, EMB], F16, tag="wenc0")
            wenc1 = wpool.tile([128, EMB], F16, tag="wenc1")
            nc.sync.dma_start(wenc0[:], wenc_d[0:128, :])
            nc.sync.dma_start(wenc1[:], wenc_d[128:256, :])
            benc = wpool.tile([1, EMB], F16, tag="benc")
            nc.sync.dma_start(benc[:], benc_d[:])
            wx = wpool.tile([EMB, CONV], F16, tag="wx")
            nc.sync.dma_start(wx[:], wx_d[:])
            wc = wpool.tile([EMB, CONV], F16, tag="wc")
            nc.sync.dma_start(wc[:], wc_d[:])
            wo = wpool.tile([CONV, 2], F16, tag="wo")
            nc.sync.dma_start(wo[:], wo_d[:])
            vecs = wpool.tile([128, 8], F32, tag="vecs")
            nc.sync.dma_start(vecs[:], vecs_d[:])
            idx = wpool.tile([128, NBLK * NIDX // 16], I16, tag="idx")
            nc.sync.dma_start(idx[:], idx_d[:])
            ones = wpool.tile([1, 128], F16, tag="ones")
            nc.vector.memset(ones[:], 1.0)

            c2v = vecs[:, 0:1]      # (2s - s^2)/8 per EMB feature
            wgv = vecs[:, 1:2]      # gn_weight
            cconv = vecs[:, 3:4]    # c_const (+cheb_b) per CONV feature
            boutv = vecs[0:1, 4:5]  # b_out scalar
            tinyv = vecs[:, 5:6]    # 1e-30
            epsv = vecs[:, 6:7]     # EPS

            # ---- encoder -> node-major token table in SBUF ----
            # table[p, r*128 + e] = clip(xenc)[node r*128+p, e]
            table = wpool.tile([128, RANKS * EMB], F16, tag="table")
            for g in range(4):
                ep = psG.tile([128, 512], F32, tag="sp", name=f"ep{g}")
                for t4 in range(4):
                    t = 4 * g + t4
                    sl = bass.ts(t, 128)
                    out = ep[:, bass.ts(t4, 128)]
                    nc.tensor.matmul(out, featT0[:, sl], wenc0[:],
                                     start=True, stop=False)
                    nc.tensor.matmul(out, featT1[:, sl], wenc1[:],
                                     start=False, stop=False)
                    nc.tensor.matmul(out, ones[:], benc[:],
                                     start=False, stop=True)
                nc.vector.tensor_scalar(
                    table[:, bass.ts(g, 512)], ep[:],
                    1.0, -1.0, op0=OP.min, op1=OP.max)

            logit = wpool.tile([1, EPAD], F32, tag="logit")

            for b in range(NBLK):
                # ---- transposed SBUF-source gathers: feature-major x rows.
                # One gather is capped at ~1000 idxs (ucode idx staging) and
                # ~125 descriptors per direction.
                # xg[p, j*L + e] = x[feat p, node(edge e, member j)]
                xg = gpool.tile([128, S * L], F16, tag="xg")
                off = 0
                for ci, n in enumerate((896, 896, 896, 896, 512)):
                    nc.gpsimd.dma_gather(
                        out_ap=xg[:, off:off + n].unsqueeze(1),
                        in_ap=table[:],
                        idxs_ap=idx[:, b * (NIDX // 16) + off // 16:
                                    b * (NIDX // 16) + (off + n) // 16],
                        num_idxs=n,
                        num_idxs_reg=n,
                        elem_size=EMB,
                        transpose=True,
                        sbuf_tokens_per_rank=128,
                        sbuf_free_dim_per_rank=2 * EMB,
                        sbuf_free_dim_pad_per_rank=0,
                        sbuf_byte_offset=0,
                        queue_num=(b * 5 + ci) % 2,
                    )
                    off += n
                xgx = xg[:].rearrange("p (j e) -> p j e", j=S)
                xsq = bigp.tile([128, S, L], F16, tag="xsq")
                nc.scalar.activation(xsq[:], xgx, AF.Square)

                # ---- per-edge sums over the 8 member planes: fp16
                # add-trees, final level in fp32.
                g1 = bigp.tile([128, 4, L], F16, tag="gq1")
                nc.vector.tensor_tensor(g1[:], xgx[:, 0:4], xgx[:, 4:8],
                                        op=OP.add)
                g2 = spool.tile([128, 2, L], F16, tag="g2")
                nc.vector.tensor_tensor(g2[:], g1[:, 0:2], g1[:, 2:4],
                                        op=OP.add)
                g8s = spool.tile([128, L], F32, tag="g8s")
                nc.vector.tensor_tensor(g8s[:], g2[:, 0], g2[:, 1], op=OP.add)
                q1 = bigp.tile([128, 4, L], F16, tag="q1")
                nc.vector.tensor_tensor(q1[:], xsq[:, 0:4], xsq[:, 4:8],
                                        op=OP.add)
                q2 = spool.tile([128, 2, L], F16, tag="q2")
                nc.vector.tensor_tensor(q2[:], q1[:, 0:2], q1[:, 2:4],
                                        op=OP.add)
                q8s = spool.tile([128, L], F32, tag="q8s")
                nc.vector.tensor_tensor(q8s[:], q2[:, 0], q2[:, 1], op=OP.add)

                # GraphNorm per-hyperedge scale A = gn_w / sqrt(var + eps)
                t1 = spool.tile([128, L], F32, tag="t1")
                nc.scalar.activation(t1[:], g8s[:], AF.Square)
                t2 = spool.tile([128, L], F32, tag="t2")
                nc.vector.tensor_scalar(t2[:], t1[:], c2v, None, op0=OP.mult)
                vx8 = spool.tile([128, L], F32, tag="vx8")
                nc.vector.tensor_tensor(vx8[:], q8s[:], t2[:], op=OP.subtract)
                ex = spool.tile([128, L], F32, tag="ex")
                nc.scalar.activation(ex[:], vx8[:], AF.Abs_reciprocal_sqrt,
                                     scale=0.125, bias=epsv)
                A16 = spool.tile([128, L], F16, tag="A16")
                nc.vector.tensor_scalar(A16[:], ex[:], wgv, None, op0=OP.mult)
                w8 = spool.tile([128, L], F16, tag="w8")
                nc.vector.tensor_tensor(w8[:], A16[:], g8s[:], op=OP.mult)

                # ---- rhs = A (.) x, broadcast A over the 8 member planes
                rhs = bigp.tile([128, S, L], F16, tag="rhs")
                nc.vector.tensor_tensor(
                    rhs[:], xgx, A16[:].unsqueeze(1).broadcast_to([128, S, L]),
                    op=OP.mult)

                # ---- cheb + per-edge C in PSUM; egress with c_const bias
                z = bigp.tile([128, S, L], F16, tag="z")
                for w in range(4):
                    vp = psV.tile([128, 2, L], F32, tag="vp", name=f"vp{b}_{w}")
                    nc.tensor.matmul(vp[:, 0, :], wx[:], rhs[:, 2 * w, :],
                                     start=True, stop=False)
                    nc.tensor.matmul(vp[:, 1, :], wx[:], rhs[:, 2 * w + 1, :],
                                     start=True, stop=False)
                    nc.tensor.matmul(vp[:, 0, :], wc[:], w8[:],
                                     start=False, stop=True)
                    nc.tensor.matmul(vp[:, 1, :], wc[:], w8[:],
                                     start=False, stop=True)
                    nc.scalar.activation(z[:, 2 * w:2 * w + 2, :], vp[:],
                                         AF.Identity, bias=cconv)

                # ---- poolings over the 8 planes (fp16 DVE trees) ----
                mx1 = bigp.tile([128, 4, L], F16, tag="mx1")
                mn1 = bigp.tile([128, 4, L], F16, tag="mn1")
                nc.vector.tensor_tensor(mx1[:], z[:, 0:4, :], z[:, 4:8, :],
                                        op=OP.max)
                nc.vector.tensor_tensor(mn1[:], z[:, 0:4, :], z[:, 4:8, :],
                                        op=OP.min)
                mx2 = spool.tile([128, 2, L], F16, tag="mx2")
                mn2 = spool.tile([128, 2, L], F16, tag="mn2")
                nc.vector.tensor_tensor(mx2[:], mx1[:, 0:2, :], mx1[:, 2:4, :],
                                        op=OP.max)
                nc.vector.tensor_tensor(mn2[:], mn1[:, 0:2, :], mn1[:, 2:4, :],
                                        op=OP.min)
                zmax = spool.tile([128, L], F16, tag="zmax")
                zmin = spool.tile([128, L], F16, tag="zmin")
                nc.vector.tensor_tensor(zmax[:], mx2[:, 0, :], mx2[:, 1, :],
                                        op=OP.max)
                nc.vector.tensor_tensor(zmin[:], mn2[:, 0, :], mn2[:, 1, :],
                                        op=OP.min)
                # rng = clip(zmax) - clip(zmin)
                zmaxc = spool.tile([128, L], F16, tag="zmaxc")
                zminc = spool.tile([128, L], F16, tag="zminc")
                nc.vector.tensor_scalar(zmaxc[:], zmax[:], 1.0, -1.0,
                                        op0=OP.min, op1=OP.max)
                nc.vector.tensor_scalar(zminc[:], zmin[:], 1.0, -1.0,
                                        op0=OP.min, op1=OP.max)
                rng = spool.tile([128, L], F16, tag="rng")
                nc.vector.tensor_tensor(rng[:], zmaxc[:], zminc[:],
                                        op=OP.subtract)

                # ---- ynorm = sqrt(mean_j min(z^2, 1)) ----
                sqm = bigp.tile([128, S, L], F16, tag="rhs", name=f"sqm{b}")
                nc.scalar.activation(sqm[:], z[:], AF.Square)
                nc.vector.tensor_scalar(sqm[:], sqm[:], 1.0, None, op0=OP.min)
                s1 = bigp.tile([128, 4, L], F16, tag="mx1", name=f"s1{b}")
                nc.vector.tensor_tensor(s1[:], sqm[:, 0:4, :], sqm[:, 4:8, :],
                                        op=OP.add)
                s2 = spool.tile([128, 2, L], F16, tag="mx2", name=f"s2{b}")
                nc.vector.tensor_tensor(s2[:], s1[:, 0:2, :], s1[:, 2:4, :],
                                        op=OP.add)
                ssq = spool.tile([128, L], F32, tag="ssq")
                nc.vector.tensor_tensor(ssq[:], s2[:, 0, :], s2[:, 1, :],
                                        op=OP.add)
                r2 = spool.tile([128, L], F32, tag="r2")
                nc.scalar.activation(r2[:], ssq[:], AF.Abs_reciprocal_sqrt,
                                     scale=0.125, bias=tinyv)
                ynorm = spool.tile([128, L], F16, tag="ynorm")
                nc.vector.scalar_tensor_tensor(ynorm[:], ssq[:], 0.125, r2[:],
                                               op0=OP.mult, op1=OP.mult)

                fp = psG.tile([1, L], F32, tag="fp", name=f"fp{b}")
                nc.tensor.matmul(fp[:], wo[:, 0:1], rng[:],
                                 start=True, stop=False)
                nc.tensor.matmul(fp[:], wo[:, 1:2], ynorm[:],
                                 start=False, stop=True)
                nc.scalar.activation(logit[0:1, bass.ts(b, L)], fp[:],
                                     AF.Identity)

            nc.scalar.activation(logit[:], logit[:], AF.Sigmoid, bias=boutv)
            nc.sync.dma_start(yout_d[:].rearrange("(p c) -> p c", p=1),
                              logit[:])

    nc.compile()
    return nc


def _get_program():
    if "nc" not in _CACHE:
        _CACHE["nc"] = _build_program()
    return _CACHE["nc"]


def _host_prep(inputs):
    """Fold weights and stage per-core input maps."""
    f = lambda k: np.asarray(inputs[k], np.float32)
    feature = f("feature")
    W_enc, b_enc = f("W_enc"), f("b_enc")
    gw, gb, gs = f("gn_weight"), f("gn_bias"), f("gn_mean_scale")
    cheb_W = np.asarray(inputs["cheb_W"], np.float64)
    cheb_b = np.asarray(inputs["cheb_b"], np.float64)
    W_out, b_out = f("W_out"), f("b_out")
    hn = np.asarray(inputs["hyperedge_nodes"]).astype(np.int64)

    d = float(S - 1)
    W0, W1, W2 = cheb_W[0], cheb_W[1], cheb_W[2]
    Wx64 = W0 + W1 / d + W2 * ((2.0 - d * d) / (d * d))
    Wg64 = -W1 / d + W2 * (2.0 * (d - 1.0) / (d * d))
    # z_j = (A.x_j) @ Wx + w8 @ WC + c_const,  w8 = A.g8
    WC64 = Wg64 - (gs.astype(np.float64)[:, None] / S) * (Wx64 + S * Wg64)
    c_const = gb.astype(np.float64) @ (Wx64 + S * Wg64) + cheb_b

    featT = np.zeros((F, NPAD), np.float16)
    featT[:, :N] = feature.T.astype(np.float16)
    wenc = W_enc.astype(np.float16)
    benc = b_enc.reshape(1, EMB).astype(np.float16)
    wx16 = Wx64.astype(np.float16)
    wc16 = WC64.astype(np.float16)
    wo16 = np.stack([W_out[:CONV, 0], W_out[CONV:, 0]], axis=1).astype(np.float16)
    eyef = np.eye(128, dtype=np.float16)
    vecs = np.zeros((128, 8), np.float32)
    vecs[:, 0] = (2.0 * gs - gs * gs) / 8.0
    vecs[:, 1] = gw
    vecs[:, 3] = c_const.astype(np.float32)
    vecs[0, 4] = b_out[0]
    vecs[:, 5] = 1e-30
    vecs[:, 6] = EPS

    shared = dict(featT=featT, wenc=wenc, benc=benc, wx=wx16, wc=wc16,
                  wo=wo16, eyef=eyef, vecs=vecs)

    in_maps = []
    for c in range(NCORES):
        base = c * ECORE
        hcol = np.zeros((EPAD, S), np.int16)
        hcol[:ECORE] = hn[base:base + ECORE].astype(np.int16)
        # per-block idx stream (plane-major: i = j*L + e), wrapped in 16
        # partitions; gather chunks slice contiguous column ranges.
        ids = np.transpose(hcol.reshape(NBLK, L, S), (0, 2, 1)).reshape(
            NBLK, NIDX)
        idxw = np.zeros((NBLK, 16, NIDX // 16), np.int16)
        pos = np.arange(NIDX)
        for b in range(NBLK):
            idxw[b, pos % 16, pos // 16] = ids[b]
        idx16 = np.tile(idxw.transpose(1, 0, 2).reshape(16, -1), (8, 1))
        in_maps.append(dict(shared, idx16=idx16))
    return in_maps


def _install_trace_hook():
    """Best-effort NTFF profiling under axon (test/benchmark only)."""
    import types
    ah = sys.modules.get("antenv.axon_hooks")
    if ah is None:
        ah = types.ModuleType("antenv.axon_hooks")
        ah._HOOK = None
        ah.set_axon_ntff_profile_hook = lambda h: setattr(ah, "_HOOK", h)
        ah.get_axon_ntff_profile_hook = lambda: ah._HOOK
        sys.modules["antenv.axon_hooks"] = ah
        import antenv
        antenv.axon_hooks = ah
    if ah.get_axon_ntff_profile_hook() is None:
        from trn_agent_boot.trn_boot import _ntff_profile_via_ctypes
        hook = _ntff_profile_via_ctypes("/opt/axon/libaxon_pjrt.so")
        if hook is not None:
            ah.set_axon_ntff_profile_hook(hook)
    import concourse.bass_utils as bu
    bu.upload_artifacts = lambda tmpdir: f"local:{tmpdir}"


def _run(in_maps, trace=False):
    nc = _get_program()
    if trace:
        _install_trace_hook()
    return run_bass_kernel_spmd(nc, in_maps, list(range(NCORES)), trace=trace)


def kernel(**inputs) -> np.ndarray:
    in_maps = _host_prep(inputs)
    res = _run(in_maps)
    out = np.concatenate([res.results[c]["yout"][:ECORE] for c in range(NCORES)])
    return out.reshape(E, 1).astype(np.float32)


def kernel_traced(**inputs):
    """Like kernel() but returns (output, exec_time_ns) using a profiled run."""
    in_maps = _host_prep(inputs)
    res = _run(in_maps, trace=True)
    out = np.concatenate([res.results[c]["yout"][:ECORE] for c in range(NCORES)])
    return out.reshape(E, 1).astype(np.float32), res.exec_time_ns


# revision 41
# speedup vs baseline: 1.6401x; 1.2139x over previous
"""Trainium2 Bass kernel for nn_CHESHIRE (hypergraph GNN message passing).

Strategy (hyperedge-parallel across the 8 cores):
  * Clique Laplacian has the closed form lap(v) = (v - gsum(v))/7, so the
    K=3 Chebyshev conv collapses to z_j = (A.x_j) @ Wx + w8 @ WC + c_const
    with host-folded weight combos (A, w8 per-hyperedge GraphNorm terms).
  * The encoder output [x || x^2] lives in SBUF as a node-major token table;
    incidence rows are fetched feature-major with ONE transposing SBUF-source
    dma_gather per 512-edge block (4096 descriptors amortize the ~1us SWDGE
    fixed cost, and the transpose removes all PE transpose traffic).
  * Per-edge sums (g8/q8) use a single accumulating identity matmul with a
    stride-0 revisit output AP; per-edge C is accumulated straight into the
    cheb PSUM the same way, so PSUM egress is a scalar-engine activation with
    a per-partition bias and the DVE never reads PSUM for the z path.
  * Max/min poolings are fp16 tensor_tensor trees on DVE; ssq pooling is
    another revisit matmul.
"""

import sys

sys.path.insert(0, "/opt/trn_rl_repo")

import numpy as np

import concourse.bacc as bacc
import concourse.bass as bass
import concourse.mybir as mybir
from concourse import tile
from concourse.bass_utils import run_bass_kernel_spmd

F16 = mybir.dt.float16
F32 = mybir.dt.float32
I16 = mybir.dt.int16
AF = mybir.ActivationFunctionType
OP = mybir.AluOpType

# Problem constants (hardcoded per contract).
N, F, EMB, CONV = 2000, 256, 128, 128
E, S = 20000, 8
NCORES = 8
ECORE = E // NCORES          # 2500
EPAD = 2560                  # padded per-core edge count
NBLK = 5
L = EPAD // NBLK             # 512 edges per block
NIDX = S * L                 # 4096 gathered rows per block
NPAD = 2048                  # padded node count (16 ranks of 128)
RANKS = NPAD // 128
EPS = 1e-5

_CACHE = {}


def _build_program():
    nc = bacc.Bacc(None, target_bir_lowering=False, debug=False,
                   num_swdge_queues=2)

    featT_d = nc.dram_tensor("featT", [F, NPAD], F16, kind="ExternalInput")
    wenc_d = nc.dram_tensor("wenc", [F, EMB], F16, kind="ExternalInput")
    benc_d = nc.dram_tensor("benc", [1, EMB], F16, kind="ExternalInput")
    wx_d = nc.dram_tensor("wx", [EMB, CONV], F16, kind="ExternalInput")
    wc_d = nc.dram_tensor("wc", [EMB, CONV], F16, kind="ExternalInput")
    wo_d = nc.dram_tensor("wo", [CONV, 2], F16, kind="ExternalInput")
    eyef_d = nc.dram_tensor("eyef", [128, 128], F16, kind="ExternalInput")
    vecs_d = nc.dram_tensor("vecs", [128, 8], F32, kind="ExternalInput")
    idx_d = nc.dram_tensor("idx16", [128, NBLK * NIDX // 16], I16,
                           kind="ExternalInput")
    yout_d = nc.dram_tensor("yout", [EPAD], F32, kind="ExternalOutput")

    with tile.TileContext(nc) as tc:
        with (
            tc.tile_pool(name="weights", bufs=1) as wpool,
            tc.tile_pool(name="smalls", bufs=2) as spool,
            tc.tile_pool(name="gath", bufs=3) as gpool,
            tc.tile_pool(name="big", bufs=2) as bigp,
            tc.tile_pool(name="psV", bufs=2, space="PSUM") as psV,
            tc.tile_pool(name="psG", bufs=1, space="PSUM") as psG,
        ):
            # ---- load weights / tables ----
            featT0 = wpool.tile([128, NPAD], F16, tag="featT0")
            featT1 = wpool.tile([128, NPAD], F16, tag="featT1")
            nc.sync.dma_start(featT0[:], featT_d[0:128, :])
            nc.sync.dma_start(featT1[:], featT_d[128:256, :])
            wenc0 = wpool.tile([128, EMB], F16, tag="wenc0")
            wenc1 = wpool.tile([128, EMB], F16, tag="wenc1")
            nc.sync.dma_start(wenc0[:], wenc_d[0:128, :])
            nc.sync.dma_start(wenc1[:], wenc_d[128:256, :])
            benc = wpool.tile([1, EMB], F16, tag="benc")
            nc.sync.dma_start(benc[:], benc_d[:])
            wx = wpool.tile([EMB, CONV], F16, tag="wx")
            nc.sync.dma_start(wx[:], wx_d[:])
            wc = wpool.tile([EMB, CONV], F16, tag="wc")
            nc.sync.dma_start(wc[:], wc_d[:])
            wo = wpool.tile([CONV, 2], F16, tag="wo")
            nc.sync.dma_start(wo[:], wo_d[:])
            vecs = wpool.tile([128, 8], F32, tag="vecs")
            nc.sync.dma_start(vecs[:], vecs_d[:])
            idx = wpool.tile([128, NBLK * NIDX // 16], I16, tag="idx")
            nc.sync.dma_start(idx[:], idx_d[:])
            ones = wpool.tile([1, 128], F16, tag="ones")
            nc.vector.memset(ones[:], 1.0)

            c2v = vecs[:, 0:1]      # (2s - s^2)/8 per EMB feature
            wgv = vecs[:, 1:2]      # gn_weight
            cconv = vecs[:, 3:4]    # c_const (+cheb_b) per CONV feature
            boutv = vecs[0:1, 4:5]  # b_out scalar
            tinyv = vecs[:, 5:6]    # 1e-30
            epsv = vecs[:, 6:7]     # EPS

            # ---- encoder -> node-major token table in SBUF ----
            # table[p, r*128 + e] = clip(xenc)[node r*128+p, e]
            table = wpool.tile([128, RANKS * EMB], F16, tag="table")
            for g in range(4):
                ep = psG.tile([128, 512], F32, tag="sp", name=f"ep{g}")
                for t4 in range(4):
                    t = 4 * g + t4
                    sl = bass.ts(t, 128)
                    out = ep[:, bass.ts(t4, 128)]
                    nc.tensor.matmul(out, featT0[:, sl], wenc0[:],
                                     start=True, stop=False)
                    nc.tensor.matmul(out, featT1[:, sl], wenc1[:],
                                     start=False, stop=False)
                    nc.tensor.matmul(out, ones[:], benc[:],
                                     start=False, stop=True)
                nc.vector.tensor_scalar(
                    table[:, bass.ts(g, 512)], ep[:],
                    1.0, -1.0, op0=OP.min, op1=OP.max)

            logit = wpool.tile([1, EPAD], F32, tag="logit")

            for b in range(NBLK):
                # ---- transposed SBUF-source gathers: feature-major x rows.
                # One gather is capped at ~1000 idxs (ucode idx staging) and
                # ~125 descriptors per direction.
                # xg[p, j*L + e] = x[feat p, node(edge e, member j)]
                xcat = gpool.tile([128, 2, S * L], F16, tag="xg")
                xg = xcat[:, 0, :]
                off = 0
                for ci, n in enumerate((896, 896, 896, 896, 512)):
                    nc.gpsimd.dma_gather(
                        out_ap=xg[:, off:off + n].unsqueeze(1),
                        in_ap=table[:],
                        idxs_ap=idx[:, b * (NIDX // 16) + off // 16:
                                    b * (NIDX // 16) + (off + n) // 16],
                        num_idxs=n,
                        num_idxs_reg=n,
                        elem_size=EMB,
                        transpose=True,
                        sbuf_tokens_per_rank=128,
                        sbuf_free_dim_per_rank=2 * EMB,
                        sbuf_free_dim_pad_per_rank=0,
                        sbuf_byte_offset=0,
                        queue_num=(b * 5 + ci) % 2,
                    )
                    off += n
                xgx = xg[:].rearrange("p (j e) -> p j e", j=S)
                xcv = xcat[:].rearrange("p h (j e) -> p h j e", j=S)
                nc.scalar.activation(xcv[:, 1], xgx, AF.Square)

                # ---- per-edge sums over the 8 member planes (x and x^2 in
                # one fused fp16 add-tree), final level in fp32.
                gq1 = bigp.tile([128, 2, 4, L], F16, tag="gq1")
                nc.vector.tensor_tensor(gq1[:], xcv[:, :, 0:4], xcv[:, :, 4:8],
                                        op=OP.add)
                gq2 = spool.tile([128, 2, 2, L], F16, tag="gq2")
                nc.vector.tensor_tensor(gq2[:], gq1[:, :, 0:2], gq1[:, :, 2:4],
                                        op=OP.add)
                gqs = spool.tile([128, 2, L], F32, tag="gqs")
                nc.vector.tensor_tensor(gqs[:], gq2[:, :, 0, :],
                                        gq2[:, :, 1, :], op=OP.add)
                g8s = gqs[:, 0, :]
                q8s = gqs[:, 1, :]

                # GraphNorm per-hyperedge scale A = gn_w / sqrt(var + eps)
                t1 = spool.tile([128, L], F32, tag="t1")
                nc.scalar.activation(t1[:], g8s, AF.Square)
                t2 = spool.tile([128, L], F32, tag="t2")
                nc.vector.tensor_scalar(t2[:], t1[:], c2v, None, op0=OP.mult)
                vx8 = spool.tile([128, L], F32, tag="vx8")
                nc.vector.tensor_tensor(vx8[:], q8s, t2[:], op=OP.subtract)
                ex = spool.tile([128, L], F32, tag="ex")
                nc.scalar.activation(ex[:], vx8[:], AF.Abs_reciprocal_sqrt,
                                     scale=0.125, bias=epsv)
                A16 = spool.tile([128, L], F16, tag="A16")
                nc.vector.tensor_scalar(A16[:], ex[:], wgv, None, op0=OP.mult)
                w8 = spool.tile([128, L], F16, tag="w8")
                nc.vector.tensor_tensor(w8[:], A16[:], g8s, op=OP.mult)

                # ---- rhs = A (.) x, broadcast A over the 8 member planes
                rhs = bigp.tile([128, S, L], F16, tag="rhs")
                nc.vector.tensor_tensor(
                    rhs[:], xgx, A16[:].unsqueeze(1).broadcast_to([128, S, L]),
                    op=OP.mult)

                # ---- cheb + per-edge C in PSUM; egress with c_const bias
                z = bigp.tile([128, S, L], F16, tag="z")
                for w in range(4):
                    vp = psV.tile([128, 2, L], F32, tag="vp", name=f"vp{b}_{w}")
                    nc.tensor.matmul(vp[:, 0, :], wx[:], rhs[:, 2 * w, :],
                                     start=True, stop=False)
                    nc.tensor.matmul(vp[:, 1, :], wx[:], rhs[:, 2 * w + 1, :],
                                     start=True, stop=False)
                    nc.tensor.matmul(vp[:, 0, :], wc[:], w8[:],
                                     start=False, stop=True)
                    nc.tensor.matmul(vp[:, 1, :], wc[:], w8[:],
                                     start=False, stop=True)
                    nc.scalar.activation(z[:, 2 * w:2 * w + 2, :], vp[:],
                                         AF.Identity, bias=cconv)

                # ---- poolings over the 8 planes (fp16 DVE trees) ----
                mx1 = bigp.tile([128, 4, L], F16, tag="mx1")
                mn1 = bigp.tile([128, 4, L], F16, tag="mn1")
                nc.vector.tensor_tensor(mx1[:], z[:, 0:4, :], z[:, 4:8, :],
                                        op=OP.max)
                nc.vector.tensor_tensor(mn1[:], z[:, 0:4, :], z[:, 4:8, :],
                                        op=OP.min)
                mx2 = spool.tile([128, 2, L], F16, tag="mx2")
                mn2 = spool.tile([128, 2, L], F16, tag="mn2")
                nc.vector.tensor_tensor(mx2[:], mx1[:, 0:2, :], mx1[:, 2:4, :],
                                        op=OP.max)
                nc.vector.tensor_tensor(mn2[:], mn1[:, 0:2, :], mn1[:, 2:4, :],
                                        op=OP.min)
                zmax = spool.tile([128, L], F16, tag="zmax")
                zmin = spool.tile([128, L], F16, tag="zmin")
                nc.vector.tensor_tensor(zmax[:], mx2[:, 0, :], mx2[:, 1, :],
                                        op=OP.max)
                nc.vector.tensor_tensor(zmin[:], mn2[:, 0, :], mn2[:, 1, :],
                                        op=OP.min)
                # rng = clip(zmax) - clip(zmin)
                zmaxc = spool.tile([128, L], F16, tag="zmaxc")
                zminc = spool.tile([128, L], F16, tag="zminc")
                nc.vector.tensor_scalar(zmaxc[:], zmax[:], 1.0, -1.0,
                                        op0=OP.min, op1=OP.max)
                nc.vector.tensor_scalar(zminc[:], zmin[:], 1.0, -1.0,
                                        op0=OP.min, op1=OP.max)
                rng = spool.tile([128, L], F16, tag="rng")
                nc.vector.tensor_tensor(rng[:], zmaxc[:], zminc[:],
                                        op=OP.subtract)

                # ---- ynorm = sqrt(mean_j min(z^2, 1)) ----
                sq0 = bigp.tile([128, S, L], F16, tag="rhs", name=f"sq0{b}")
                nc.scalar.activation(sq0[:], z[:], AF.Square)
                sqm = bigp.tile([128, S, L], F16, tag="z", name=f"sqm{b}")
                nc.vector.tensor_scalar(sqm[:], sq0[:], 1.0, None, op0=OP.min)
                s1 = bigp.tile([128, 4, L], F16, tag="mx1", name=f"s1{b}")
                nc.vector.tensor_tensor(s1[:], sqm[:, 0:4, :], sqm[:, 4:8, :],
                                        op=OP.add)
                s2 = spool.tile([128, 2, L], F16, tag="mx2", name=f"s2{b}")
                nc.vector.tensor_tensor(s2[:], s1[:, 0:2, :], s1[:, 2:4, :],
                                        op=OP.add)
                ssq = spool.tile([128, L], F32, tag="ssq")
                nc.vector.tensor_tensor(ssq[:], s2[:, 0, :], s2[:, 1, :],
                                        op=OP.add)
                r2 = spool.tile([128, L], F32, tag="r2")
                nc.scalar.activation(r2[:], ssq[:], AF.Abs_reciprocal_sqrt,
                                     scale=0.125, bias=tinyv)
                ynorm = spool.tile([128, L], F16, tag="ynorm")
                nc.vector.scalar_tensor_tensor(ynorm[:], ssq[:], 0.125, r2[:],
                                               op0=OP.mult, op1=OP.mult)

                fp = psG.tile([1, L], F32, tag="fp", name=f"fp{b}")
                nc.tensor.matmul(fp[:], wo[:, 0:1], rng[:],
                                 start=True, stop=False)
                nc.tensor.matmul(fp[:], wo[:, 1:2], ynorm[:],
                                 start=False, stop=True)
                nc.scalar.activation(logit[0:1, bass.ts(b, L)], fp[:],
                                     AF.Identity)

            nc.scalar.activation(logit[:], logit[:], AF.Sigmoid, bias=boutv)
            nc.sync.dma_start(yout_d[:].rearrange("(p c) -> p c", p=1),
                              logit[:])

    nc.compile()
    return nc


def _get_program():
    if "nc" not in _CACHE:
        _CACHE["nc"] = _build_program()
    return _CACHE["nc"]


def _host_prep(inputs):
    """Fold weights and stage per-core input maps."""
    f = lambda k: np.asarray(inputs[k], np.float32)
    feature = f("feature")
    W_enc, b_enc = f("W_enc"), f("b_enc")
    gw, gb, gs = f("gn_weight"), f("gn_bias"), f("gn_mean_scale")
    cheb_W = np.asarray(inputs["cheb_W"], np.float64)
    cheb_b = np.asarray(inputs["cheb_b"], np.float64)
    W_out, b_out = f("W_out"), f("b_out")
    hn = np.asarray(inputs["hyperedge_nodes"]).astype(np.int64)

    d = float(S - 1)
    W0, W1, W2 = cheb_W[0], cheb_W[1], cheb_W[2]
    Wx64 = W0 + W1 / d + W2 * ((2.0 - d * d) / (d * d))
    Wg64 = -W1 / d + W2 * (2.0 * (d - 1.0) / (d * d))
    # z_j = (A.x_j) @ Wx + w8 @ WC + c_const,  w8 = A.g8
    WC64 = Wg64 - (gs.astype(np.float64)[:, None] / S) * (Wx64 + S * Wg64)
    c_const = gb.astype(np.float64) @ (Wx64 + S * Wg64) + cheb_b

    featT = np.zeros((F, NPAD), np.float16)
    featT[:, :N] = feature.T.astype(np.float16)
    wenc = W_enc.astype(np.float16)
    benc = b_enc.reshape(1, EMB).astype(np.float16)
    wx16 = Wx64.astype(np.float16)
    wc16 = WC64.astype(np.float16)
    wo16 = np.stack([W_out[:CONV, 0], W_out[CONV:, 0]], axis=1).astype(np.float16)
    eyef = np.eye(128, dtype=np.float16)
    vecs = np.zeros((128, 8), np.float32)
    vecs[:, 0] = (2.0 * gs - gs * gs) / 8.0
    vecs[:, 1] = gw
    vecs[:, 3] = c_const.astype(np.float32)
    vecs[0, 4] = b_out[0]
    vecs[:, 5] = 1e-30
    vecs[:, 6] = EPS

    shared = dict(featT=featT, wenc=wenc, benc=benc, wx=wx16, wc=wc16,
                  wo=wo16, eyef=eyef, vecs=vecs)

    in_maps = []
    for c in range(NCORES):
        base = c * ECORE
        hcol = np.zeros((EPAD, S), np.int16)
        hcol[:ECORE] = hn[base:base + ECORE].astype(np.int16)
        # per-block idx stream (plane-major: i = j*L + e), wrapped in 16
        # partitions; gather chunks slice contiguous column ranges.
        ids = np.transpose(hcol.reshape(NBLK, L, S), (0, 2, 1)).reshape(
            NBLK, NIDX)
        idxw = np.zeros((NBLK, 16, NIDX // 16), np.int16)
        pos = np.arange(NIDX)
        for b in range(NBLK):
            idxw[b, pos % 16, pos // 16] = ids[b]
        idx16 = np.tile(idxw.transpose(1, 0, 2).reshape(16, -1), (8, 1))
        in_maps.append(dict(shared, idx16=idx16))
    return in_maps


def _install_trace_hook():
    """Best-effort NTFF profiling under axon (test/benchmark only)."""
    import types
    ah = sys.modules.get("antenv.axon_hooks")
    if ah is None:
        ah = types.ModuleType("antenv.axon_hooks")
        ah._HOOK = None
        ah.set_axon_ntff_profile_hook = lambda h: setattr(ah, "_HOOK", h)
        ah.get_axon_ntff_profile_hook = lambda: ah._HOOK
        sys.modules["antenv.axon_hooks"] = ah
        import antenv
        antenv.axon_hooks = ah
    if ah.get_axon_ntff_profile_hook() is None:
        from trn_agent_boot.trn_boot import _ntff_profile_via_ctypes
        hook = _ntff_profile_via_ctypes("/opt/axon/libaxon_pjrt.so")
        if hook is not None:
            ah.set_axon_ntff_profile_hook(hook)
    import concourse.bass_utils as bu
    bu.upload_artifacts = lambda tmpdir: f"local:{tmpdir}"


def _run(in_maps, trace=False):
    nc = _get_program()
    if trace:
        _install_trace_hook()
    return run_bass_kernel_spmd(nc, in_maps, list(range(NCORES)), trace=trace)


def kernel(**inputs) -> np.ndarray:
    in_maps = _host_prep(inputs)
    res = _run(in_maps)
    out = np.concatenate([res.results[c]["yout"][:ECORE] for c in range(NCORES)])
    return out.reshape(E, 1).astype(np.float32)


def kernel_traced(**inputs):
    """Like kernel() but returns (output, exec_time_ns) using a profiled run."""
    in_maps = _host_prep(inputs)
    res = _run(in_maps, trace=True)
    out = np.concatenate([res.results[c]["yout"][:ECORE] for c in range(NCORES)])
    return out.reshape(E, 1).astype(np.float32), res.exec_time_ns
